# revision 19
# baseline (speedup 1.0000x reference)
"""Distributed Trainium2 (Bass/Tile) kernel for a causal self-attention block.

Reference computation (per batch b):
    qk = x_eps @ W_eps_attn ; q,k = split(qk) ; vp = v @ W_attn
    q,k = rope(q), rope(k)   (llama-style, 16 heads x 128 dims)
    y   = causal_softmax(q k^T / sqrt(128)) @ vp   (per head)
    v_out     = y @ W_proj
    x_eps_out = x_eps @ W_eps_proj

Sharding over 8 NeuronCores: core = (b, g) = 4-way batch x 2-way head-group
(8 heads per core).  W_eps_attn/W_attn are column-sharded by head; y is
exchanged pair-wise per 512-token half (AllGather overlapped with the second
attention half / x_eps projection) and W_proj/W_eps_proj are used
column-sharded so each core produces a disjoint half of both outputs.

Phase order per core:  B (q/k proj + RoPE) -> C (vp proj) -> D (attention)
-> X/V output projections (collectives hidden under X).  All matmuls bf16
with fp32 PSUM accumulation.

Scheduling notes baked into the structure:
 - ONE psum layout for the whole kernel: a [128,1024] pair pool (2 tiles,
   4 banks) + a [128,512] quad pool (4 tiles, 4 banks).  Every stage draws
   its accumulators from these two pools, so stage handoffs are tile-level
   WAR deps instead of pool-zone barriers (kills the C->D and D->X gaps);
 - B alternates its 4 accumulators between the quad pool (even G) and
   pair halves (odd G) so group G's RoPE reads have all of G+1 to drain
   before G+2 overwrites the slots; C and X/V run tile-major so their 8
   accumulators complete staggered and the drains never gate the PE;
 - stage D is triangle-restricted: score/y/z matmuls only cover keys
   <= query tile, causal masking is done in PSUM by accumulating a
   [128,128] -60000 upper-triangular tile (one 128-cycle matmul per
   diagonal block) so exp(masked) = 0 exactly and the DVE mask multiplies
   disappear; exp reads a whole score-pair [128, <=1024] PSUM AP in one
   ACTIVATE (short tiles packed into one bank) to amortize the ~352-cycle
   ACT instruction overhead; the head loop is software-pipelined one unit
   deep (two during qc0) and runs continuously across the qc boundary;
 - the PE warm-up chain is long enough (~24 matmuls) to cover the G0/G1
   input-DMA bandwidth deficit, so the PE never idles into a HAM
   down-throttle during the ramp (a 1-2us stall costs ~4us more in
   half-clock hangover);
 - stage C streams W_attn through the same 5-slot weight pool as B's wqk
   groups (tile-level FIFO handoff), issued as half-chunk descriptors;
 - RoPE does 4 PSUM-reading multiplies on VectorE + 2 SBUF folds on GpSimdE
   (no PSUM->SBUF staging copies);
 - softmax denominator via a single ones-matmul per k-tile (replicated over
   partitions by the PE) and reciprocal_approx_fast;
 - outputs are written bf16 (host upcasts) halving the output DMA.
"""

import sys

sys.path.insert(0, "/opt/trn_rl_repo")

import numpy as np
import ml_dtypes

import concourse.bass as bass
import concourse.mybir as mybir
import concourse.tile as tile
from concourse import bacc
from concourse.bass_utils import run_bass_kernel_spmd

F32 = mybir.dt.float32
BF16 = mybir.dt.bfloat16
BF16_NP = ml_dtypes.bfloat16

B, T, DIM, H, HD = 4, 1024, 2048, 16, 128
NCORES = 8
HL = H // 2          # heads per core (8)
TT = T // 128        # token tiles (8)
CC = DIM // 128      # contraction chunks (16)
QC = T // 512        # 512-token halves (2)
COLS = DIM // 2      # local output columns (1024)

_COMPILED = None


def _build():
    nc = bacc.Bacc(trn_type="TRN2", target_bir_lowering=False, debug=False,
                   num_devices=NCORES)

    MUL = mybir.AluOpType.mult
    SUB = mybir.AluOpType.subtract
    ADD = mybir.AluOpType.add

    # ---- per-core I/O: everything partition-major [128, n*free] ----
    x_in = nc.dram_tensor("x_in", [128, CC * T], BF16, kind="ExternalInput").ap()
    v_in = nc.dram_tensor("v_in", [128, CC * T], BF16, kind="ExternalInput").ap()
    cos_in = nc.dram_tensor("cosd", [128, T], BF16, kind="ExternalInput").ap()
    sin_in = nc.dram_tensor("sind", [128, T], BF16, kind="ExternalInput").ap()
    wqk_in = nc.dram_tensor("w_qk", [8, 128, CC * 256], BF16,
                            kind="ExternalInput").ap()
    wat_in = nc.dram_tensor("w_attn", [128, CC * COLS], BF16,
                            kind="ExternalInput").ap()
    wpr_in = nc.dram_tensor("w_proj", [128, CC * COLS], BF16,
                            kind="ExternalInput").ap()
    wep_in = nc.dram_tensor("w_eps_proj", [128, CC * COLS], BF16,
                            kind="ExternalInput").ap()
    v_out = nc.dram_tensor("v_out", [128, TT * COLS], BF16,
                           kind="ExternalOutput").ap()
    x_out = nc.dram_tensor("x_out", [128, TT * COLS], BF16,
                           kind="ExternalOutput").ap()

    # internal DRAM for the chunked pair-wise y exchange
    y_bounce = [nc.dram_tensor(f"y_bounce{qc}", [128, HL * 512], BF16)
                for qc in range(QC)]
    y_gather = [nc.dram_tensor(f"y_gather{qc}", [2, 128, HL * 512], BF16)
                for qc in range(QC)]

    with tile.TileContext(nc) as tc:
        with tc.tile_pool(name="pp", bufs=1) as pp, \
             tc.tile_pool(name="rp", bufs=2) as rp, \
             tc.tile_pool(name="ps2", bufs=2, space="PSUM") as ps2, \
             tc.tile_pool(name="ps1", bufs=4, space="PSUM") as ps1:

            def pair_tile(name):
                return ps2.tile([128, 1024], F32, tag="pgrp", name=name)

            def quad_tile(name):
                return ps1.tile([128, 512], F32, tag="pq", name=name)

            # 8 x [128,512] accumulators for the dense GEMM stages: 4 from
            # the quad pool + halves of 2 pair tiles, as (tile, col_base)
            # pairs.  Allocation order = slot cycling order, so consecutive
            # stages hand off per tile (no pool-zone barriers).
            def gemm_accums(name):
                q = [(quad_tile(f"{name}q{i}"), 0) for i in range(4)]
                p = [pair_tile(f"{name}p{i}") for i in range(2)]
                return q + [(p[0], 0), (p[0], 512), (p[1], 0), (p[1], 512)]

            def quad_accums(name):
                return [(quad_tile(f"{name}q{i}"), 0) for i in range(4)]

            def pair_accums(name):
                p = [pair_tile(f"{name}p{i}") for i in range(2)]
                return [(p[0], 0), (p[0], 512), (p[1], 0), (p[1], 512)]

            def acc_ap(acc, rows=slice(0, 128), lo=0, hi=512):
                t, base = acc
                return t[rows, base + lo:base + hi]

            xT = pp.tile([128, CC * T], BF16, tag="xT", name="xT")
            wprT = pp.tile([128, CC * COLS], BF16, tag="wpr", name="wprT")
            # vT doubles as the W_eps_proj buffer once stage C has drained it
            vT = pp.tile([128, CC * T], BF16, tag="vT", name="vT")
            ones_mat = pp.tile([128, 128], BF16, tag="ones", name="ones_mat")
            nc.vector.memset(ones_mat[:], 1.0)

            # first thing on the sync HWDGE queue: the chunk stage B needs
            nc.sync.dma_start(xT[:, 0:T], x_in[:, 0:T])

            # PE warm-up: ~3.5us of matmul activity while the first input
            # DMAs are in flight trips the HAM clock gate to 8/8 before the
            # real work starts (the DMA to scratch keeps it from being DCEd)
            wu_scratch = nc.dram_tensor("wu_scratch", [128, 512], F32)
            wu_in = pp.tile([128, 512], BF16, tag="wu_in", name="wu_in")
            nc.vector.memset(wu_in[:], 0.0)

            with tc.tile_pool(name="bdp", bufs=1) as bdp:
                qkT = bdp.tile([128, 2 * HL * T], BF16, tag="qkT", name="qkT")
                vp = bdp.tile([128, TT * COLS], BF16, tag="vp", name="vp")
                # causal helpers: identity (stationary) and the strict upper
                # triangular -60000 tile added into diagonal score blocks
                ident = bdp.tile([128, 128], BF16, tag="ident", name="ident")
                nc.gpsimd.memset(ident[:], 1.0)
                nc.gpsimd.affine_select(
                    out=ident[:], in_=ident[:], compare_op=mybir.AluOpType.is_ge,
                    fill=0.0, base=0, pattern=[[1, 128]], channel_multiplier=-1)
                nc.gpsimd.affine_select(
                    out=ident[:], in_=ident[:], compare_op=mybir.AluOpType.is_ge,
                    fill=0.0, base=0, pattern=[[-1, 128]], channel_multiplier=1)
                negm = bdp.tile([128, 128], BF16, tag="negm", name="negm")
                nc.gpsimd.memset(negm[:], 0.0)
                nc.gpsimd.affine_select(
                    out=negm[:], in_=negm[:], compare_op=mybir.AluOpType.is_ge,
                    fill=-60000.0, base=0, pattern=[[1, 128]],
                    channel_multiplier=-1)

                with tc.tile_pool(name="wp", bufs=5) as wp, \
                     tc.tile_pool(name="rtp", bufs=2) as rtp, \
                     tc.tile_pool(name="cstp", bufs=1) as cstp:
                    # shared weight-stream pool: B's wqk groups and C's
                    # W_attn chunks are the same [128,4096] shape, so one
                    # 5-slot pool gives tile-level FIFO handoff B->C
                    def wtile(name):
                        return wp.tile([128, CC * 256], BF16, tag="wbig",
                                       name=name)

                    # ---- stage B: q/k projection + RoPE ----
                    if True:

                        # cos/sin duplicated into both partition halves so every
                        # RoPE multiply pairs equal base partitions
                        cosD = cstp.tile([128, T], BF16, tag="cosD", name="cosD")
                        nc.gpsimd.dma_start(cosD[:], cos_in)
                        sinD = cstp.tile([128, T], BF16, tag="sinD", name="sinD")
                        nc.gpsimd.dma_start(sinD[:], sin_in)

                        # long enough to cover the G0/G1 input-DMA deficit so
                        # the PE never idles into a HAM down-throttle mid-ramp
                        pwu = pair_tile("pwu")
                        for i in range(24):
                            nc.tensor.matmul(pwu[:, 0:512], ones_mat[:], wu_in[:],
                                             start=(i == 0), stop=(i == 23))
                        wu_out = rtp.tile([128, 512], F32, tag="wu_out")
                        nc.vector.tensor_copy(wu_out[:], pwu[:, 0:512])
                        nc.sync.dma_start(wu_scratch.ap(), wu_out[:])

                        for G in range(8):
                            wt = wtile(f"wqk{G}")
                            if G == 0:
                                # interleave G0's weight/activation pieces so
                                # the bandwidth-limited ramp stalls stay small
                                nc.sync.dma_start(wt[:, 0:512], wqk_in[0][:, 0:512])
                                nc.sync.dma_start(xT[:, T:3 * T], x_in[:, T:3 * T])
                                nc.sync.dma_start(wt[:, 512:1024],
                                                  wqk_in[0][:, 512:1024])
                                nc.sync.dma_start(xT[:, 3 * T:6 * T],
                                                  x_in[:, 3 * T:6 * T])
                                nc.sync.dma_start(wt[:, 1024:2048],
                                                  wqk_in[0][:, 1024:2048])
                                nc.sync.dma_start(xT[:, 6 * T:10 * T],
                                                  x_in[:, 6 * T:10 * T])
                                nc.sync.dma_start(wt[:, 2048:3072],
                                                  wqk_in[0][:, 2048:3072])
                                nc.sync.dma_start(xT[:, 10 * T:13 * T],
                                                  x_in[:, 10 * T:13 * T])
                                nc.sync.dma_start(wt[:, 3072:],
                                                  wqk_in[0][:, 3072:])
                                nc.sync.dma_start(xT[:, 13 * T:], x_in[:, 13 * T:])
                            else:
                                nc.sync.dma_start(wt[:], wqk_in[G])
                                # vT arrives mid-late B (only C consumes it),
                                # keeping the early queue clear for wqk
                                if G in (4, 5, 6):
                                    ib = (G - 4) * 5 * T
                                    ie = CC * T if G == 6 else ib + 5 * T
                                    s = slice(ib, ie)
                                    nc.sync.dma_start(vT[:, s], v_in[:, s])

                            # alternate accumulator banks per G so the RoPE
                            # reads of group G have all of G+1 to drain before
                            # G+2 overwrites the slots
                            acc = (quad_accums(f"psB{G}_") if G % 2 == 0
                                   else pair_accums(f"psB{G}_"))
                            psg = [[acc[2 * dl + q] for q in range(QC)]
                                   for dl in range(2)]
                            for c in range(CC):
                                for dl in range(2):
                                    w_sl = wt[:, c * 256 + dl * 128:
                                              c * 256 + (dl + 1) * 128]
                                    for q in range(QC):
                                        nc.tensor.matmul(
                                            acc_ap(psg[dl][q]), w_sl,
                                            xT[:, c * T + q * 512:
                                               c * T + q * 512 + 512],
                                            start=(c == 0), stop=(c == CC - 1))
                            # RoPE: psum rows 0:64 = re, 64:128 = im
                            for dl in range(2):
                                dt = 2 * G + dl
                                for q in range(QC):
                                    ps_re = acc_ap(psg[dl][q], slice(0, 64))
                                    ps_im = acc_ap(psg[dl][q], slice(64, 128))
                                    cs = slice(q * 512, (q + 1) * 512)
                                    oc = slice(dt * T + q * 512,
                                               dt * T + q * 512 + 512)
                                    t1 = rtp.tile([64, 512], BF16, tag="t1")
                                    nc.vector.tensor_tensor(
                                        t1[:], ps_re, cosD[0:64, cs], MUL)
                                    t2 = rtp.tile([64, 512], BF16, tag="t2")
                                    nc.vector.tensor_tensor(
                                        t2[:], ps_im, sinD[64:128, cs], MUL)
                                    nc.gpsimd.tensor_tensor(
                                        qkT[0:64, oc], t1[:], t2[:], SUB)
                                    t3 = rtp.tile([64, 512], BF16, tag="t3")
                                    nc.vector.tensor_tensor(
                                        t3[:], ps_re, sinD[0:64, cs], MUL)
                                    t4 = rtp.tile([64, 512], BF16, tag="t4")
                                    nc.vector.tensor_tensor(
                                        t4[:], ps_im, cosD[64:128, cs], MUL)
                                    nc.gpsimd.tensor_tensor(
                                        qkT[64:128, oc], t3[:], t4[:], ADD)

                    # ---- stage C: vp = v @ W_attn (both token halves) ----
                    # tile-major (tl outer) so the 8 accumulators complete
                    # staggered and their drains never block the next group
                    if True:
                        wtiles = [None] * 8
                        def issue_wat(idx):
                            cb = idx % 4  # W_attn is streamed once per half
                            wtb = wtile(f"wat{idx}")
                            # two half-chunk descriptors: consumers of the
                            # first two c-slices unblock ~1.4us earlier
                            for hf in range(2):
                                s = slice(hf * 2 * COLS, (hf + 1) * 2 * COLS)
                                nc.sync.dma_start(
                                    wtb[:, s],
                                    wat_in[:, cb * 4 * COLS + hf * 2 * COLS:
                                           cb * 4 * COLS + (hf + 1) * 2 * COLS])
                            wtiles[idx] = wtb
                        for idx in range(4):
                            issue_wat(idx)
                        for qcv in range(QC):
                            acc = gemm_accums(f"psC{qcv}_")
                            psg = [[acc[2 * tl + dh] for dh in range(2)]
                                   for tl in range(4)]
                            for tl in range(4):
                                t = qcv * 4 + tl
                                for cb in range(4):
                                    wtb = wtiles[qcv * 4 + cb]
                                    for ci in range(4):
                                        c = cb * 4 + ci
                                        v_sl = vT[:, c * T + t * 128:
                                                  c * T + t * 128 + 128]
                                        for dh in range(2):
                                            nc.tensor.matmul(
                                                acc_ap(psg[tl][dh]), v_sl,
                                                wtb[:, ci * COLS + dh * 512:
                                                    ci * COLS + dh * 512 + 512],
                                                start=(c == 0),
                                                stop=(c == CC - 1))
                                # drain split across DVE/ACT right after each
                                # tile finishes
                                for dh in range(2):
                                    dst = vp[:, t * COLS + dh * 512:
                                             t * COLS + dh * 512 + 512]
                                    if dh == 0:
                                        nc.vector.tensor_copy(
                                            dst, acc_ap(psg[tl][dh]))
                                    else:
                                        nc.scalar.copy(
                                            dst, acc_ap(psg[tl][dh]))
                                if qcv == 0:
                                    issue_wat(4 + tl)

                # queue W_proj and W_eps_proj behind the attention weights:
                # both transfer during stage D, before the collectives start
                # competing for DMA bandwidth.  W_eps_proj reuses vT's space
                # (stage C has fully consumed v by now).
                half = CC * COLS // 2
                nc.sync.dma_start(wprT[:, 0:half], wpr_in[:, 0:half])
                nc.sync.dma_start(wprT[:, half:], wpr_in[:, half:])
                nc.sync.dma_start(vT[:, 0:half], wep_in[:, 0:half])
                nc.sync.dma_start(vT[:, half:], wep_in[:, half:])

                # ---- stage D: causal attention, triangle-restricted ----
                # score/y/z matmuls cover only keys<=query-tile; causal mask
                # applied in PSUM via ident.T @ negm accumulate; one wide exp
                # per score pair.  Emission is software-pipelined one head
                # deep (scores of head j+1 before y/z of head j) so the PE
                # never waits on exp latency: score-pair slots are freed by
                # exp (emitted in the same A-phase), py/pz quads by the
                # normalize (B-phases stay FIFO), so the WAR chain is clean.
                with tc.tile_pool(name="ytbp", bufs=2) as ytbp, \
                     tc.tile_pool(name="ptp", bufs=8) as ptp:
                    ytb = [ytbp.tile([128, HL * 512], BF16, tag="ytb",
                                     name=f"ytb{qc}") for qc in range(QC)]

                    def qlo(qc, ki):
                        # restricted query-window start per k-tile
                        return max(0, 128 * ki - qc * 512)

                    def emit_scores(qc, j, prange=None):
                        K = 4 * (qc + 1)
                        kbase = (HL + j) * T
                        qbase = j * T + qc * 512
                        info = {}
                        for p in (range(K // 2) if prange is None else prange):
                            grp = pair_tile(f"sg{qc}_{j}_{p}")
                            ka, kb = 2 * p, 2 * p + 1
                            wa = 512 - qlo(qc, ka)
                            wb = 512 - qlo(qc, kb)
                            # pack the second tile right after the first when
                            # both fit one PSUM bank: shortens the exp read
                            pos = {ka: 0, kb: wa if wa + wb <= 512 else 512}
                            rd = pos[kb] + wb
                            pt = ptp.tile([128, 1024], BF16, tag="pt",
                                          name=f"pt{qc}_{j}_{p}")
                            for ki in (ka, kb):
                                lo = qlo(qc, ki)
                                cb = pos[ki]
                                diag = ki >= 4 * qc
                                nc.tensor.matmul(
                                    grp[:, cb:cb + 512 - lo],
                                    qkT[:, kbase + ki * 128:
                                        kbase + (ki + 1) * 128],
                                    qkT[:, qbase + lo: qbase + 512],
                                    start=True, stop=not diag)
                                if diag:
                                    nc.tensor.matmul(
                                        grp[:, cb:cb + 128],
                                        ident[:], negm[:],
                                        start=False, stop=True)
                                info[ki] = (pt, cb)
                            nc.scalar.activation(
                                pt[:, 0:rd], grp[:, 0:rd],
                                mybir.ActivationFunctionType.Exp)
                        return info

                    def emit_yz(qc, j, info):
                        K = 4 * (qc + 1)
                        py = quad_tile(f"py{qc}_{j}")
                        pz = quad_tile(f"pz{qc}_{j}")
                        for ki in range(K):
                            lo = qlo(qc, ki)
                            pt, cb = info[ki]
                            src = pt[:, cb:cb + 512 - lo]
                            nc.tensor.matmul(
                                py[:, lo:512],
                                vp[:, ki * COLS + j * 128:
                                   ki * COLS + (j + 1) * 128],
                                src, start=(ki == 0), stop=(ki == K - 1))
                            nc.tensor.matmul(
                                pz[:, lo:512], ones_mat[:], src,
                                start=(ki == 0), stop=(ki == K - 1))
                        zr = rp.tile([128, 512], F32, tag="zr",
                                     name=f"zr{qc}_{j}")
                        nc.vector.reciprocal_approx_fast(zr[:], pz[:])
                        nc.vector.tensor_tensor(
                            ytb[qc][:, j * 512:(j + 1) * 512],
                            py[:], zr[:], MUL)

                    def ship(qc):
                        # own 8 heads -> DRAM -> pair AllGather
                        nc.gpsimd.dma_start(y_bounce[qc].ap(), ytb[qc][:])
                        nc.gpsimd.collective_compute(
                            "AllGather", mybir.AluOpType.bypass,
                            replica_groups=[[0, 1], [2, 3], [4, 5], [6, 7]],
                            ins=[y_bounce[qc].ap()], outs=[y_gather[qc].ap()])

                    # one continuous software pipeline across both halves so
                    # the PE never drains at the qc boundary; 2 units deep in
                    # qc0 (hides first-exp latency; qc0 units are small), 1
                    # deep in qc1 (pt pool peaks at exactly 8 tiles)
                    units = [(qc, j) for qc in range(QC) for j in range(HL)]
                    pend = []

                    def flush_one():
                        (fqc, fj), info = pend.pop(0)
                        emit_yz(fqc, fj, info)
                        if fj == HL - 1:
                            ship(fqc)

                    for u in units:
                        depth = 2 if u[0] == 0 else 1
                        pend.append((u, emit_scores(*u)))
                        while len(pend) > depth:
                            flush_one()
                    while pend:
                        flush_one()

            # ---- stages X/V: output projections (exchange hidden under X) ----
            with tc.tile_pool(name="xvp", bufs=2) as xvp, \
                 tc.tile_pool(name="ocp", bufs=2) as ocp:

                yG2 = [xvp.tile([128, 2 * HL * 512], BF16, tag="yg",
                                name=f"yg{qc}") for qc in range(QC)]
                for qc in range(QC):
                    for r in range(2):
                        nc.gpsimd.dma_start(
                            yG2[qc][:, r * HL * 512:(r + 1) * HL * 512],
                            y_gather[qc].ap()[r])

                def drain_tl(psg, og, out_ap, tg, tl):
                    # casts split across DVE/ACT; per-half stores so the
                    # output DMA starts as soon as each cast lands
                    for dh in range(2):
                        dst = og[:, tl * COLS + dh * 512:
                                 tl * COLS + dh * 512 + 512]
                        if dh == 0:
                            nc.vector.tensor_copy(dst, acc_ap(psg[tl][dh]))
                        else:
                            nc.scalar.copy(dst, acc_ap(psg[tl][dh]))
                        nc.sync.dma_start(
                            out_ap[:, (tg * 4 + tl) * COLS + dh * 512:
                                   (tg * 4 + tl) * COLS + dh * 512 + 512],
                            dst)

                # tile-major (tl outer) keeps accumulator completion
                # staggered so drains never gate the next group's matmuls
                for tg in range(QC):
                    # x_eps_out rows [tg*512, ..+512): no exchange dependency
                    acc = gemm_accums(f"psX{tg}_")
                    psg = [[acc[2 * tl + dh] for dh in range(2)]
                           for tl in range(4)]
                    og = ocp.tile([128, 4 * COLS], BF16, tag="og", name=f"ox{tg}")
                    for tl in range(4):
                        t = tg * 4 + tl
                        for c in range(CC):
                            x_sl = xT[:, c * T + t * 128: c * T + t * 128 + 128]
                            for dh in range(2):
                                nc.tensor.matmul(
                                    acc_ap(psg[tl][dh]), x_sl,
                                    vT[:, c * COLS + dh * 512:
                                       c * COLS + dh * 512 + 512],
                                    start=(c == 0), stop=(c == CC - 1))
                        drain_tl(psg, og, x_out, tg, tl)

                    # v_out rows for the same token half (needs exchange tg)
                    acc = gemm_accums(f"psV{tg}_")
                    psg = [[acc[2 * tl + dh] for dh in range(2)]
                           for tl in range(4)]
                    og = ocp.tile([128, 4 * COLS], BF16, tag="og", name=f"ov{tg}")
                    for tl in range(4):
                        for c in range(CC):
                            y_sl = yG2[tg][:, c * 512 + tl * 128:
                                           c * 512 + tl * 128 + 128]
                            for dh in range(2):
                                nc.tensor.matmul(
                                    acc_ap(psg[tl][dh]), y_sl,
                                    wprT[:, c * COLS + dh * 512:
                                         c * COLS + dh * 512 + 512],
                                    start=(c == 0), stop=(c == CC - 1))
                        drain_tl(psg, og, v_out, tg, tl)

    nc.compile()
    return nc


def _prep_core_inputs(inputs):
    """Host-side shard prep: slicing, bf16 cast, partition-major packing."""
    x_eps = np.asarray(inputs["x_eps"], np.float32)
    v = np.asarray(inputs["v"], np.float32)
    cos = np.asarray(inputs["freqs_cos"], np.float32)
    sin = np.asarray(inputs["freqs_sin"], np.float32)
    Wqk = np.asarray(inputs["W_eps_attn"], np.float32)
    Wat = np.asarray(inputs["W_attn"], np.float32)
    Wpr = np.asarray(inputs["W_proj"], np.float32)
    Wep = np.asarray(inputs["W_eps_proj"], np.float32)

    cosD = np.ascontiguousarray(
        np.concatenate([cos.T, cos.T], axis=0)).astype(BF16_NP)   # [128, T]
    sinD = np.ascontiguousarray(
        np.concatenate([sin.T, sin.T], axis=0)).astype(BF16_NP)
    perm = np.concatenate([np.arange(0, HD, 2), np.arange(1, HD, 2)])  # re|im
    scale = np.float32(1.0 / np.sqrt(HD))

    def pm_act(a):  # [T, DIM] fp32 -> [128, CC*T] bf16, col = c*T + t
        return np.ascontiguousarray(
            a.astype(BF16_NP).T.reshape(CC, 128, T)
            .transpose(1, 0, 2).reshape(128, CC * T))

    xT_bf = [pm_act(x_eps[b]) for b in range(B)]
    vT_bf = [pm_act(v[b]) for b in range(B)]

    def pm_w(Wc):  # [DIM, COLS] fp32 -> [128, CC*COLS] bf16, col = c*COLS + f
        return np.ascontiguousarray(
            Wc.reshape(CC, 128, COLS).transpose(1, 0, 2)
            .reshape(128, CC * COLS).astype(BF16_NP))

    per_g = []
    for g in range(2):
        heads = range(g * HL, (g + 1) * HL)
        wq = np.concatenate(
            [Wqk[:, h * HD:(h + 1) * HD][:, perm] * scale for h in heads], axis=1)
        wk = np.concatenate(
            [Wqk[:, DIM + h * HD:DIM + (h + 1) * HD][:, perm] for h in heads],
            axis=1)
        wqk_cols = np.concatenate([wq, wk], axis=1)          # [2048, 2048]
        # [G, p, c*256 + dl*128 + col]
        wqk_p = np.ascontiguousarray(
            wqk_cols.reshape(CC, 128, 8, 2, 128)
            .transpose(2, 1, 0, 3, 4).reshape(8, 128, CC * 256)).astype(BF16_NP)
        cols = slice(g * COLS, (g + 1) * COLS)
        per_g.append({
            "w_qk": wqk_p,
            "w_attn": pm_w(Wat[:, cols]),
            "w_proj": pm_w(Wpr[:, cols]),
            "w_eps_proj": pm_w(Wep[:, cols]),
        })

    in_maps = []
    for core in range(NCORES):
        b, g = divmod(core, 2)
        in_maps.append({
            "x_in": xT_bf[b],
            "v_in": vT_bf[b],
            "cosd": cosD,
            "sind": sinD,
            **per_g[g],
        })
    return in_maps


def _get_compiled():
    global _COMPILED
    if _COMPILED is None:
        _COMPILED = _build()
    return _COMPILED


def kernel(**inputs):
    nc = _get_compiled()
    in_maps = _prep_core_inputs(inputs)
    res = run_bass_kernel_spmd(nc, in_maps, list(range(NCORES)))
    v_full = np.empty((B, T, DIM), np.float32)
    x_full = np.empty((B, T, DIM), np.float32)
    for core in range(NCORES):
        b, g = divmod(core, 2)
        r = res.results[core]
        cols = slice(g * COLS, (g + 1) * COLS)
        vo = np.asarray(r["v_out"]).reshape(128, TT, COLS).transpose(1, 0, 2)
        xo = np.asarray(r["x_out"]).reshape(128, TT, COLS).transpose(1, 0, 2)
        v_full[b][:, cols] = vo.reshape(T, COLS).astype(np.float32)
        x_full[b][:, cols] = xo.reshape(T, COLS).astype(np.float32)
    return (v_full, x_full)


# revision 23
# speedup vs baseline: 1.0045x; 1.0045x over previous
"""Distributed Trainium2 (Bass/Tile) kernel for a causal self-attention block.

Reference computation (per batch b):
    qk = x_eps @ W_eps_attn ; q,k = split(qk) ; vp = v @ W_attn
    q,k = rope(q), rope(k)   (llama-style, 16 heads x 128 dims)
    y   = causal_softmax(q k^T / sqrt(128)) @ vp   (per head)
    v_out     = y @ W_proj
    x_eps_out = x_eps @ W_eps_proj

Sharding over 8 NeuronCores: core = (b, g) = 4-way batch x 2-way head-group
(8 heads per core).  W_eps_attn/W_attn are column-sharded by head; y is
exchanged pair-wise per 512-token half (AllGather overlapped with the second
attention half / x_eps projection) and W_proj/W_eps_proj are used
column-sharded so each core produces a disjoint half of both outputs.

Phase order per core:  B (q/k proj + RoPE) -> C (vp proj) -> D (attention)
-> X/V output projections (collectives hidden under X).  All matmuls bf16
with fp32 PSUM accumulation.

Scheduling notes baked into the structure:
 - ONE psum layout for the whole kernel: a [128,1024] pair pool (2 tiles,
   4 banks) + a [128,512] quad pool (4 tiles, 4 banks).  Every stage draws
   its accumulators from these two pools, so stage handoffs are tile-level
   WAR deps instead of pool-zone barriers (kills the C->D and D->X gaps);
 - B alternates its 4 accumulators between the quad pool (even G) and
   pair halves (odd G) so group G's RoPE reads have all of G+1 to drain
   before G+2 overwrites the slots; C and X/V run tile-major so their 8
   accumulators complete staggered and the drains never gate the PE;
 - stage D is triangle-restricted: score/y/z matmuls only cover keys
   <= query tile, causal masking is done in PSUM by accumulating a
   [128,128] -60000 upper-triangular tile (one 128-cycle matmul per
   diagonal block) so exp(masked) = 0 exactly and the DVE mask multiplies
   disappear; exp reads a whole score-pair [128, <=1024] PSUM AP in one
   ACTIVATE (short tiles packed into one bank) to amortize the ~352-cycle
   ACT instruction overhead; the head loop is software-pipelined one unit
   deep (two during qc0) and runs continuously across the qc boundary;
 - the PE warm-up chain is long enough (~24 matmuls) to cover the G0/G1
   input-DMA bandwidth deficit, so the PE never idles into a HAM
   down-throttle during the ramp (a 1-2us stall costs ~4us more in
   half-clock hangover);
 - stage C streams W_attn through the same 5-slot weight pool as B's wqk
   groups (tile-level FIFO handoff), issued as half-chunk descriptors;
 - RoPE does 4 PSUM-reading multiplies on VectorE + 2 SBUF folds on GpSimdE
   (no PSUM->SBUF staging copies);
 - softmax denominator via a single ones-matmul per k-tile (replicated over
   partitions by the PE) and reciprocal_approx_fast;
 - outputs are written bf16 (host upcasts) halving the output DMA.
"""

import sys

sys.path.insert(0, "/opt/trn_rl_repo")

import numpy as np
import ml_dtypes

import concourse.bass as bass
import concourse.mybir as mybir
import concourse.tile as tile
from concourse import bacc
from concourse.bass_utils import run_bass_kernel_spmd

F32 = mybir.dt.float32
BF16 = mybir.dt.bfloat16
BF16_NP = ml_dtypes.bfloat16

B, T, DIM, H, HD = 4, 1024, 2048, 16, 128
NCORES = 8
HL = H // 2          # heads per core (8)
TT = T // 128        # token tiles (8)
CC = DIM // 128      # contraction chunks (16)
QC = T // 512        # 512-token halves (2)
COLS = DIM // 2      # local output columns (1024)

_COMPILED = None


def _build():
    nc = bacc.Bacc(trn_type="TRN2", target_bir_lowering=False, debug=False,
                   num_devices=NCORES)

    MUL = mybir.AluOpType.mult
    SUB = mybir.AluOpType.subtract
    ADD = mybir.AluOpType.add

    # ---- per-core I/O: everything partition-major [128, n*free] ----
    x_in = nc.dram_tensor("x_in", [128, CC * T], BF16, kind="ExternalInput").ap()
    v_in = nc.dram_tensor("v_in", [128, CC * T], BF16, kind="ExternalInput").ap()
    cos_in = nc.dram_tensor("cosd", [128, T], BF16, kind="ExternalInput").ap()
    sin_in = nc.dram_tensor("sind", [128, T], BF16, kind="ExternalInput").ap()
    wqk_in = nc.dram_tensor("w_qk", [8, 128, CC * 256], BF16,
                            kind="ExternalInput").ap()
    wat_in = nc.dram_tensor("w_attn", [128, CC * COLS], BF16,
                            kind="ExternalInput").ap()
    wpr_in = nc.dram_tensor("w_proj", [128, CC * COLS], BF16,
                            kind="ExternalInput").ap()
    wep_in = nc.dram_tensor("w_eps_proj", [128, CC * COLS], BF16,
                            kind="ExternalInput").ap()
    v_out = nc.dram_tensor("v_out", [128, TT * COLS], BF16,
                           kind="ExternalOutput").ap()
    x_out = nc.dram_tensor("x_out", [128, TT * COLS], BF16,
                           kind="ExternalOutput").ap()

    # internal DRAM for the chunked pair-wise y exchange
    y_bounce = [nc.dram_tensor(f"y_bounce{qc}", [128, HL * 512], BF16)
                for qc in range(QC)]
    y_gather = [nc.dram_tensor(f"y_gather{qc}", [2, 128, HL * 512], BF16)
                for qc in range(QC)]

    with tile.TileContext(nc) as tc:
        with tc.tile_pool(name="pp", bufs=1) as pp, \
             tc.tile_pool(name="rp", bufs=2) as rp, \
             tc.tile_pool(name="ps2", bufs=2, space="PSUM") as ps2, \
             tc.tile_pool(name="ps1", bufs=4, space="PSUM") as ps1:

            def pair_tile(name):
                return ps2.tile([128, 1024], F32, tag="pgrp", name=name)

            def quad_tile(name):
                return ps1.tile([128, 512], F32, tag="pq", name=name)

            # 8 x [128,512] accumulators for the dense GEMM stages: 4 from
            # the quad pool + halves of 2 pair tiles, as (tile, col_base)
            # pairs.  Allocation order = slot cycling order, so consecutive
            # stages hand off per tile (no pool-zone barriers).
            def gemm_accums(name):
                q = [(quad_tile(f"{name}q{i}"), 0) for i in range(4)]
                p = [pair_tile(f"{name}p{i}") for i in range(2)]
                return q + [(p[0], 0), (p[0], 512), (p[1], 0), (p[1], 512)]

            def quad_accums(name):
                return [(quad_tile(f"{name}q{i}"), 0) for i in range(4)]

            def pair_accums(name):
                p = [pair_tile(f"{name}p{i}") for i in range(2)]
                return [(p[0], 0), (p[0], 512), (p[1], 0), (p[1], 512)]

            def acc_ap(acc, rows=slice(0, 128), lo=0, hi=512):
                t, base = acc
                return t[rows, base + lo:base + hi]

            xT = pp.tile([128, CC * T], BF16, tag="xT", name="xT")
            wprT = pp.tile([128, CC * COLS], BF16, tag="wpr", name="wprT")
            # vT doubles as the W_eps_proj buffer once stage C has drained it
            vT = pp.tile([128, CC * T], BF16, tag="vT", name="vT")
            ones_mat = pp.tile([128, 128], BF16, tag="ones", name="ones_mat")
            nc.vector.memset(ones_mat[:], 1.0)

            # first thing on the sync HWDGE queue: the chunk stage B needs
            nc.sync.dma_start(xT[:, 0:T], x_in[:, 0:T])

            # PE warm-up: ~3.5us of matmul activity while the first input
            # DMAs are in flight trips the HAM clock gate to 8/8 before the
            # real work starts (the DMA to scratch keeps it from being DCEd)
            wu_scratch = nc.dram_tensor("wu_scratch", [128, 512], F32)
            wu_in = pp.tile([128, 512], BF16, tag="wu_in", name="wu_in")
            nc.vector.memset(wu_in[:], 0.0)

            with tc.tile_pool(name="bdp", bufs=1) as bdp:
                qkT = bdp.tile([128, 2 * HL * T], BF16, tag="qkT", name="qkT")
                vp = bdp.tile([128, TT * COLS], BF16, tag="vp", name="vp")
                # causal helpers: identity (stationary) and the strict upper
                # triangular -60000 tile added into diagonal score blocks
                ident = bdp.tile([128, 128], BF16, tag="ident", name="ident")
                nc.gpsimd.memset(ident[:], 1.0)
                nc.gpsimd.affine_select(
                    out=ident[:], in_=ident[:], compare_op=mybir.AluOpType.is_ge,
                    fill=0.0, base=0, pattern=[[1, 128]], channel_multiplier=-1)
                nc.gpsimd.affine_select(
                    out=ident[:], in_=ident[:], compare_op=mybir.AluOpType.is_ge,
                    fill=0.0, base=0, pattern=[[-1, 128]], channel_multiplier=1)
                negm = bdp.tile([128, 128], BF16, tag="negm", name="negm")
                nc.gpsimd.memset(negm[:], 0.0)
                nc.gpsimd.affine_select(
                    out=negm[:], in_=negm[:], compare_op=mybir.AluOpType.is_ge,
                    fill=-60000.0, base=0, pattern=[[1, 128]],
                    channel_multiplier=-1)

                with tc.tile_pool(name="wp", bufs=3) as wp, \
                     tc.tile_pool(name="rtp", bufs=2) as rtp, \
                     tc.tile_pool(name="cstp", bufs=1) as cstp:
                    # B's wqk weight-stream pool (triple-buffered)
                    def wtile(name):
                        return wp.tile([128, CC * 256], BF16, tag="wbig",
                                       name=name)

                    # ---- stage B: q/k projection + RoPE ----
                    if True:

                        # cos/sin duplicated into both partition halves so every
                        # RoPE multiply pairs equal base partitions
                        cosD = cstp.tile([128, T], BF16, tag="cosD", name="cosD")
                        nc.gpsimd.dma_start(cosD[:], cos_in)
                        sinD = cstp.tile([128, T], BF16, tag="sinD", name="sinD")
                        nc.gpsimd.dma_start(sinD[:], sin_in)

                        # long enough to cover the G0/G1 input-DMA deficit so
                        # the PE never idles into a HAM down-throttle mid-ramp
                        pwu = pair_tile("pwu")
                        for i in range(24):
                            nc.tensor.matmul(pwu[:, 0:512], ones_mat[:], wu_in[:],
                                             start=(i == 0), stop=(i == 23))
                        wu_out = rtp.tile([128, 512], F32, tag="wu_out")
                        nc.vector.tensor_copy(wu_out[:], pwu[:, 0:512])
                        nc.sync.dma_start(wu_scratch.ap(), wu_out[:])

                        for G in range(8):
                            wt = wtile(f"wqk{G}")
                            if G == 0:
                                # interleave G0's weight/activation pieces so
                                # the bandwidth-limited ramp stalls stay small
                                nc.sync.dma_start(wt[:, 0:512], wqk_in[0][:, 0:512])
                                nc.sync.dma_start(xT[:, T:3 * T], x_in[:, T:3 * T])
                                nc.sync.dma_start(wt[:, 512:1024],
                                                  wqk_in[0][:, 512:1024])
                                nc.sync.dma_start(xT[:, 3 * T:6 * T],
                                                  x_in[:, 3 * T:6 * T])
                                nc.sync.dma_start(wt[:, 1024:2048],
                                                  wqk_in[0][:, 1024:2048])
                                nc.sync.dma_start(xT[:, 6 * T:10 * T],
                                                  x_in[:, 6 * T:10 * T])
                                nc.sync.dma_start(wt[:, 2048:3072],
                                                  wqk_in[0][:, 2048:3072])
                                nc.sync.dma_start(xT[:, 10 * T:13 * T],
                                                  x_in[:, 10 * T:13 * T])
                                nc.sync.dma_start(wt[:, 3072:],
                                                  wqk_in[0][:, 3072:])
                                nc.sync.dma_start(xT[:, 13 * T:], x_in[:, 13 * T:])
                            else:
                                nc.sync.dma_start(wt[:], wqk_in[G])
                                # W_attn parks in wprT (idle until V, when
                                # W_proj overwrites it); arrives over G1-G3
                                if G in (1, 2, 3):
                                    ws = [0, 5462, 10923, 16384]
                                    s = slice(ws[G - 1], ws[G])
                                    nc.sync.dma_start(wprT[:, s], wat_in[:, s])
                                # vT arrives mid-late B (only C consumes it),
                                # keeping the early queue clear for wqk
                                if G in (4, 5, 6, 7):
                                    ib = (G - 4) * 4 * T
                                    s = slice(ib, ib + 4 * T)
                                    nc.sync.dma_start(vT[:, s], v_in[:, s])

                            # alternate accumulator banks per G so the RoPE
                            # reads of group G have all of G+1 to drain before
                            # G+2 overwrites the slots
                            acc = (quad_accums(f"psB{G}_") if G % 2 == 0
                                   else pair_accums(f"psB{G}_"))
                            psg = [[acc[2 * dl + q] for q in range(QC)]
                                   for dl in range(2)]
                            for c in range(CC):
                                for dl in range(2):
                                    w_sl = wt[:, c * 256 + dl * 128:
                                              c * 256 + (dl + 1) * 128]
                                    for q in range(QC):
                                        nc.tensor.matmul(
                                            acc_ap(psg[dl][q]), w_sl,
                                            xT[:, c * T + q * 512:
                                               c * T + q * 512 + 512],
                                            start=(c == 0), stop=(c == CC - 1))
                            # RoPE: psum rows 0:64 = re, 64:128 = im
                            for dl in range(2):
                                dt = 2 * G + dl
                                for q in range(QC):
                                    ps_re = acc_ap(psg[dl][q], slice(0, 64))
                                    ps_im = acc_ap(psg[dl][q], slice(64, 128))
                                    cs = slice(q * 512, (q + 1) * 512)
                                    oc = slice(dt * T + q * 512,
                                               dt * T + q * 512 + 512)
                                    t1 = rtp.tile([64, 512], BF16, tag="t1")
                                    nc.vector.tensor_tensor(
                                        t1[:], ps_re, cosD[0:64, cs], MUL)
                                    t2 = rtp.tile([64, 512], BF16, tag="t2")
                                    nc.vector.tensor_tensor(
                                        t2[:], ps_im, sinD[64:128, cs], MUL)
                                    nc.gpsimd.tensor_tensor(
                                        qkT[0:64, oc], t1[:], t2[:], SUB)
                                    t3 = rtp.tile([64, 512], BF16, tag="t3")
                                    nc.vector.tensor_tensor(
                                        t3[:], ps_re, sinD[0:64, cs], MUL)
                                    t4 = rtp.tile([64, 512], BF16, tag="t4")
                                    nc.vector.tensor_tensor(
                                        t4[:], ps_im, cosD[64:128, cs], MUL)
                                    nc.gpsimd.tensor_tensor(
                                        qkT[64:128, oc], t3[:], t4[:], ADD)

                    # ---- stage C: vp = v @ W_attn (both token halves) ----
                    # W_attn is fully resident in wprT, so C is one smooth
                    # tile-major loop with zero DMA dependencies; each token
                    # tile's 2 accumulators free 3 tile-periods before reuse
                    if True:
                        for t in range(TT):
                            if t % 4 < 2:
                                psg = [(quad_tile(f"psC{t}_{dh}"), 0)
                                       for dh in range(2)]
                            else:
                                p = pair_tile(f"psC{t}")
                                psg = [(p, 0), (p, 512)]
                            for c in range(CC):
                                v_sl = vT[:, c * T + t * 128:
                                          c * T + t * 128 + 128]
                                for dh in range(2):
                                    nc.tensor.matmul(
                                        acc_ap(psg[dh]), v_sl,
                                        wprT[:, c * COLS + dh * 512:
                                             c * COLS + dh * 512 + 512],
                                        start=(c == 0), stop=(c == CC - 1))
                            # drain split across DVE/ACT right after each
                            # tile finishes
                            for dh in range(2):
                                dst = vp[:, t * COLS + dh * 512:
                                         t * COLS + dh * 512 + 512]
                                if dh == 0:
                                    nc.vector.tensor_copy(dst, acc_ap(psg[dh]))
                                else:
                                    nc.scalar.copy(dst, acc_ap(psg[dh]))

                # queue W_proj and W_eps_proj behind the attention weights:
                # both transfer during stage D, before the collectives start
                # competing for DMA bandwidth.  W_eps_proj reuses vT's space
                # (stage C has fully consumed v by now).
                half = CC * COLS // 2
                nc.sync.dma_start(wprT[:, 0:half], wpr_in[:, 0:half])
                nc.sync.dma_start(wprT[:, half:], wpr_in[:, half:])
                nc.sync.dma_start(vT[:, 0:half], wep_in[:, 0:half])
                nc.sync.dma_start(vT[:, half:], wep_in[:, half:])

                # ---- stage D: causal attention, triangle-restricted ----
                # score/y/z matmuls cover only keys<=query-tile; causal mask
                # applied in PSUM via ident.T @ negm accumulate; one wide exp
                # per score pair.  Emission is software-pipelined one head
                # deep (scores of head j+1 before y/z of head j) so the PE
                # never waits on exp latency: score-pair slots are freed by
                # exp (emitted in the same A-phase), py/pz quads by the
                # normalize (B-phases stay FIFO), so the WAR chain is clean.
                with tc.tile_pool(name="ytbp", bufs=2) as ytbp, \
                     tc.tile_pool(name="ptp", bufs=8) as ptp:
                    ytb = [ytbp.tile([128, HL * 512], BF16, tag="ytb",
                                     name=f"ytb{qc}") for qc in range(QC)]

                    def qlo(qc, ki):
                        # restricted query-window start per k-tile
                        return max(0, 128 * ki - qc * 512)

                    def emit_scores(qc, j, prange=None):
                        K = 4 * (qc + 1)
                        kbase = (HL + j) * T
                        qbase = j * T + qc * 512
                        info = {}
                        for p in (range(K // 2) if prange is None else prange):
                            grp = pair_tile(f"sg{qc}_{j}_{p}")
                            ka, kb = 2 * p, 2 * p + 1
                            wa = 512 - qlo(qc, ka)
                            wb = 512 - qlo(qc, kb)
                            # pack the second tile right after the first when
                            # both fit one PSUM bank: shortens the exp read
                            pos = {ka: 0, kb: wa if wa + wb <= 512 else 512}
                            rd = pos[kb] + wb
                            pt = ptp.tile([128, 1024], BF16, tag="pt",
                                          name=f"pt{qc}_{j}_{p}")
                            for ki in (ka, kb):
                                lo = qlo(qc, ki)
                                cb = pos[ki]
                                diag = ki >= 4 * qc
                                nc.tensor.matmul(
                                    grp[:, cb:cb + 512 - lo],
                                    qkT[:, kbase + ki * 128:
                                        kbase + (ki + 1) * 128],
                                    qkT[:, qbase + lo: qbase + 512],
                                    start=True, stop=not diag)
                                if diag:
                                    nc.tensor.matmul(
                                        grp[:, cb:cb + 128],
                                        ident[:], negm[:],
                                        start=False, stop=True)
                                info[ki] = (pt, cb)
                            nc.scalar.activation(
                                pt[:, 0:rd], grp[:, 0:rd],
                                mybir.ActivationFunctionType.Exp)
                        return info

                    def emit_yz(qc, j, info):
                        K = 4 * (qc + 1)
                        py = quad_tile(f"py{qc}_{j}")
                        pz = quad_tile(f"pz{qc}_{j}")
                        for ki in range(K):
                            lo = qlo(qc, ki)
                            pt, cb = info[ki]
                            src = pt[:, cb:cb + 512 - lo]
                            nc.tensor.matmul(
                                py[:, lo:512],
                                vp[:, ki * COLS + j * 128:
                                   ki * COLS + (j + 1) * 128],
                                src, start=(ki == 0), stop=(ki == K - 1))
                            nc.tensor.matmul(
                                pz[:, lo:512], ones_mat[:], src,
                                start=(ki == 0), stop=(ki == K - 1))
                        zr = rp.tile([128, 512], F32, tag="zr",
                                     name=f"zr{qc}_{j}")
                        nc.vector.reciprocal_approx_fast(zr[:], pz[:])
                        nc.vector.tensor_tensor(
                            ytb[qc][:, j * 512:(j + 1) * 512],
                            py[:], zr[:], MUL)

                    def ship(qc):
                        # own 8 heads -> DRAM -> pair AllGather
                        nc.gpsimd.dma_start(y_bounce[qc].ap(), ytb[qc][:])
                        nc.gpsimd.collective_compute(
                            "AllGather", mybir.AluOpType.bypass,
                            replica_groups=[[0, 1], [2, 3], [4, 5], [6, 7]],
                            ins=[y_bounce[qc].ap()], outs=[y_gather[qc].ap()])

                    # one continuous software pipeline across both halves so
                    # the PE never drains at the qc boundary; 2 units deep in
                    # qc0 (hides first-exp latency; qc0 units are small), 1
                    # deep in qc1 (pt pool peaks at exactly 8 tiles)
                    units = [(qc, j) for qc in range(QC) for j in range(HL)]
                    pend = []

                    def flush_one():
                        (fqc, fj), info = pend.pop(0)
                        emit_yz(fqc, fj, info)
                        if fj == HL - 1:
                            ship(fqc)

                    for u in units:
                        depth = 2 if u[0] == 0 else 1
                        pend.append((u, emit_scores(*u)))
                        while len(pend) > depth:
                            flush_one()
                    while pend:
                        flush_one()

            # ---- stages X/V: output projections (exchange hidden under X) ----
            with tc.tile_pool(name="xvp", bufs=2) as xvp, \
                 tc.tile_pool(name="ocp", bufs=2) as ocp:

                yG2 = [xvp.tile([128, 2 * HL * 512], BF16, tag="yg",
                                name=f"yg{qc}") for qc in range(QC)]
                for qc in range(QC):
                    for r in range(2):
                        nc.gpsimd.dma_start(
                            yG2[qc][:, r * HL * 512:(r + 1) * HL * 512],
                            y_gather[qc].ap()[r])

                def drain_tl(psg, og, out_ap, tg, tl):
                    # casts split across DVE/ACT; per-half stores so the
                    # output DMA starts as soon as each cast lands
                    for dh in range(2):
                        dst = og[:, tl * COLS + dh * 512:
                                 tl * COLS + dh * 512 + 512]
                        if dh == 0:
                            nc.vector.tensor_copy(dst, acc_ap(psg[tl][dh]))
                        else:
                            nc.scalar.copy(dst, acc_ap(psg[tl][dh]))
                        nc.sync.dma_start(
                            out_ap[:, (tg * 4 + tl) * COLS + dh * 512:
                                   (tg * 4 + tl) * COLS + dh * 512 + 512],
                            dst)

                # tile-major (tl outer) keeps accumulator completion
                # staggered so drains never gate the next group's matmuls
                for tg in range(QC):
                    # x_eps_out rows [tg*512, ..+512): no exchange dependency
                    acc = gemm_accums(f"psX{tg}_")
                    psg = [[acc[2 * tl + dh] for dh in range(2)]
                           for tl in range(4)]
                    og = ocp.tile([128, 4 * COLS], BF16, tag="og", name=f"ox{tg}")
                    for tl in range(4):
                        t = tg * 4 + tl
                        for c in range(CC):
                            x_sl = xT[:, c * T + t * 128: c * T + t * 128 + 128]
                            for dh in range(2):
                                nc.tensor.matmul(
                                    acc_ap(psg[tl][dh]), x_sl,
                                    vT[:, c * COLS + dh * 512:
                                       c * COLS + dh * 512 + 512],
                                    start=(c == 0), stop=(c == CC - 1))
                        drain_tl(psg, og, x_out, tg, tl)

                    # v_out rows for the same token half (needs exchange tg)
                    acc = gemm_accums(f"psV{tg}_")
                    psg = [[acc[2 * tl + dh] for dh in range(2)]
                           for tl in range(4)]
                    og = ocp.tile([128, 4 * COLS], BF16, tag="og", name=f"ov{tg}")
                    for tl in range(4):
                        for c in range(CC):
                            y_sl = yG2[tg][:, c * 512 + tl * 128:
                                           c * 512 + tl * 128 + 128]
                            for dh in range(2):
                                nc.tensor.matmul(
                                    acc_ap(psg[tl][dh]), y_sl,
                                    wprT[:, c * COLS + dh * 512:
                                         c * COLS + dh * 512 + 512],
                                    start=(c == 0), stop=(c == CC - 1))
                        drain_tl(psg, og, v_out, tg, tl)

    nc.compile()
    return nc


def _prep_core_inputs(inputs):
    """Host-side shard prep: slicing, bf16 cast, partition-major packing."""
    x_eps = np.asarray(inputs["x_eps"], np.float32)
    v = np.asarray(inputs["v"], np.float32)
    cos = np.asarray(inputs["freqs_cos"], np.float32)
    sin = np.asarray(inputs["freqs_sin"], np.float32)
    Wqk = np.asarray(inputs["W_eps_attn"], np.float32)
    Wat = np.asarray(inputs["W_attn"], np.float32)
    Wpr = np.asarray(inputs["W_proj"], np.float32)
    Wep = np.asarray(inputs["W_eps_proj"], np.float32)

    cosD = np.ascontiguousarray(
        np.concatenate([cos.T, cos.T], axis=0)).astype(BF16_NP)   # [128, T]
    sinD = np.ascontiguousarray(
        np.concatenate([sin.T, sin.T], axis=0)).astype(BF16_NP)
    perm = np.concatenate([np.arange(0, HD, 2), np.arange(1, HD, 2)])  # re|im
    scale = np.float32(1.0 / np.sqrt(HD))

    def pm_act(a):  # [T, DIM] fp32 -> [128, CC*T] bf16, col = c*T + t
        return np.ascontiguousarray(
            a.astype(BF16_NP).T.reshape(CC, 128, T)
            .transpose(1, 0, 2).reshape(128, CC * T))

    xT_bf = [pm_act(x_eps[b]) for b in range(B)]
    vT_bf = [pm_act(v[b]) for b in range(B)]

    def pm_w(Wc):  # [DIM, COLS] fp32 -> [128, CC*COLS] bf16, col = c*COLS + f
        return np.ascontiguousarray(
            Wc.reshape(CC, 128, COLS).transpose(1, 0, 2)
            .reshape(128, CC * COLS).astype(BF16_NP))

    per_g = []
    for g in range(2):
        heads = range(g * HL, (g + 1) * HL)
        wq = np.concatenate(
            [Wqk[:, h * HD:(h + 1) * HD][:, perm] * scale for h in heads], axis=1)
        wk = np.concatenate(
            [Wqk[:, DIM + h * HD:DIM + (h + 1) * HD][:, perm] for h in heads],
            axis=1)
        wqk_cols = np.concatenate([wq, wk], axis=1)          # [2048, 2048]
        # [G, p, c*256 + dl*128 + col]
        wqk_p = np.ascontiguousarray(
            wqk_cols.reshape(CC, 128, 8, 2, 128)
            .transpose(2, 1, 0, 3, 4).reshape(8, 128, CC * 256)).astype(BF16_NP)
        cols = slice(g * COLS, (g + 1) * COLS)
        per_g.append({
            "w_qk": wqk_p,
            "w_attn": pm_w(Wat[:, cols]),
            "w_proj": pm_w(Wpr[:, cols]),
            "w_eps_proj": pm_w(Wep[:, cols]),
        })

    in_maps = []
    for core in range(NCORES):
        b, g = divmod(core, 2)
        in_maps.append({
            "x_in": xT_bf[b],
            "v_in": vT_bf[b],
            "cosd": cosD,
            "sind": sinD,
            **per_g[g],
        })
    return in_maps


def _get_compiled():
    global _COMPILED
    if _COMPILED is None:
        _COMPILED = _build()
    return _COMPILED


def kernel(**inputs):
    nc = _get_compiled()
    in_maps = _prep_core_inputs(inputs)
    res = run_bass_kernel_spmd(nc, in_maps, list(range(NCORES)))
    v_full = np.empty((B, T, DIM), np.float32)
    x_full = np.empty((B, T, DIM), np.float32)
    for core in range(NCORES):
        b, g = divmod(core, 2)
        r = res.results[core]
        cols = slice(g * COLS, (g + 1) * COLS)
        vo = np.asarray(r["v_out"]).reshape(128, TT, COLS).transpose(1, 0, 2)
        xo = np.asarray(r["x_out"]).reshape(128, TT, COLS).transpose(1, 0, 2)
        v_full[b][:, cols] = vo.reshape(T, COLS).astype(np.float32)
        x_full[b][:, cols] = xo.reshape(T, COLS).astype(np.float32)
    return (v_full, x_full)
